# revision 1
# baseline (speedup 1.0000x reference)
"""CorrespondenceGeneration kernel for 8 TRN2 NeuronCores.

Reference computation (per item): unit-normalize features over channels,
build 3x3 patch matrices, corr = inp_patches^T @ ref_patches, argmax over
ref patches (first occurrence on ties), convert argmax index to flow,
9 tensor-shifts, channel reorder.

Sharding: core c -> (item = c//4, n_in chunk = c%4 of 2209 rows). Each core
computes its corr rows against ALL ref patches, streamed in 5 column groups
(widths 2048,2048,2048,2048,644 -- exactly 8836 real columns).

Engine split per (block, group) unit:
  - Tensor: fp8-e4m3 DoubleRow matmuls (two 128-row K chunks per
    instruction at 0.5 cyc/col -> 3 instructions cover K=640+zeros).
  - Vector: quarter-fold max straight out of PSUM (pm[j] = max_q
    psum[j + q*w/4], 3 tensor_max ops = 0.75 pass), then max8 +
    find_index8 over the 4x-smaller pm stream. ~1.25 full passes
    instead of 2; no scalar copy at all.
The host expands each winning (group, j) into the 4 fold-mate ref columns
and resolves them with exact dot products -- fold-internal near-ties are
decided exactly on the host and need no flagging. Rows whose cross-fold
device margin is below MARGIN_THRESH (fp8 matmul error scale) get a full
exact rescore.

Note: the reference's per-patch-column normalization of ref divides every
column by ||col||+eps with ||col|| == 3 exactly (9 unit-norm pixels), a
global positive scale that argmax is invariant to -- so it is skipped.
"""

import sys

if "/opt/trn_rl_repo" not in sys.path:
    sys.path.insert(0, "/opt/trn_rl_repo")

import numpy as np
import ml_dtypes

# ---- problem constants (hardcoded; kernel.py must be self-contained) ----
N_ITEMS = 2
C = 64
H = W = 96
PS = 3
HP = WP = H - PS + 1          # 94
NPATCH = HP * WP              # 8836
K = C * PS * PS               # 576
KPAD = 640                    # 5 x 128
KCH = 5                       # K chunks of 128
KSL = 6                       # SBUF k slots: 5 data + 1 zero (DoubleRow pad)
N_CORES = 8
CHUNKS_PER_ITEM = 4
CHUNK = NPATCH // CHUNKS_PER_ITEM      # 2209
CHUNK_PAD = 2304                       # 18 x 128
N_BLOCKS = CHUNK_PAD // 128            # 18
# ref column groups: exactly the 8836 real columns
GROUP_BASES = (0, 2048, 4096, 6144, 8192)
GROUP_WIDTHS = (2048, 2048, 2048, 2048, 644)
GROUP_QUARTERS = tuple(w // 4 for w in GROUP_WIDTHS)   # 512,512,512,512,161
N_GROUPS = len(GROUP_BASES)
# matmul strip widths per group (PSUM bank = 512 fp32; a matmul output must
# not cross a bank boundary, so strips are 512-aligned)
GROUP_STRIPS = ((512, 512, 512, 512),) * 4 + ((512, 132),)
EPS_NORMALIZE = 1e-12

# fp8-e4m3 matmul error: measured sigma ~1.4e-2, max ~8e-2 per corr entry.
# Rows whose device top-2 margin is below this get an exact host rescore.
MARGIN_THRESH = 0.07

_COMPILED = {}


def _build_module():
    import concourse.bacc as bacc
    from concourse.tile import TileContext
    from concourse import mybir

    dt_mm = mybir.dt.float8e4
    nc = bacc.Bacc("TRN2", target_bir_lowering=False, debug=False,
                   num_devices=N_CORES)
    inp_d = nc.dram_tensor("inp", [KCH, 128, CHUNK_PAD], dt_mm,
                           kind="ExternalInput").ap()
    ref_d = nc.dram_tensor("ref", [KCH, 128, NPATCH], dt_mm,
                           kind="ExternalInput").ap()
    NSLOT = N_BLOCKS * N_GROUPS            # 90
    val_d = nc.dram_tensor("val", [128, NSLOT * 8], mybir.dt.float32,
                           kind="ExternalOutput").ap()
    idx_d = nc.dram_tensor("idx", [128, NSLOT * 8], mybir.dt.uint32,
                           kind="ExternalOutput").ap()

    DR = mybir.MatmulPerfMode.DoubleRow

    with TileContext(nc) as tc:
        with tc.tile_pool(name="inp", bufs=1) as inp_pool, \
             tc.tile_pool(name="ref", bufs=2) as ref_pool, \
             tc.tile_pool(name="pm", bufs=4) as pm_pool, \
             tc.tile_pool(name="acc", bufs=1) as acc_pool, \
             tc.tile_pool(name="psum", bufs=2, space="PSUM") as psum_pool:
            # startup DMA order mirrors first-group matmul order so the first
            # matmuls are gated on ~100KB of DMA: strip-major for group 0.
            ref_tiles = {}
            w0 = GROUP_WIDTHS[0]
            ref_tiles[0] = ref_pool.tile([128, KSL, w0], dt_mm,
                                         tag="ref", name="ref_sb0")
            inp_sb = inp_pool.tile([128, KSL, CHUNK_PAD], dt_mm)
            nc.gpsimd.memset(inp_sb[:, KCH, :], 0)
            nc.gpsimd.memset(ref_tiles[0][:, KCH, :], 0)
            for k in range(KCH):
                nc.sync.dma_start(inp_sb[:, k, 0:128], inp_d[k, :, 0:128])
            for j in range(w0 // 512):
                for k in range(KCH):
                    nc.sync.dma_start(
                        ref_tiles[0][:, k, j * 512:(j + 1) * 512],
                        ref_d[k, :, j * 512:(j + 1) * 512])
            # rest of inp (group 0 iterates over all blocks, so all of inp is
            # needed early)
            for (lo, hi) in [(128, 384), (384, 896), (896, 1664),
                             (1664, CHUNK_PAD)]:
                for k in range(KCH):
                    nc.sync.dma_start(inp_sb[:, k, lo:hi], inp_d[k, :, lo:hi])
            acc_val = acc_pool.tile([128, NSLOT * 8], mybir.dt.float32)
            acc_idx = acc_pool.tile([128, NSLOT * 8], mybir.dt.uint32)

            units = [(s, b) for s in range(N_GROUPS)
                     for b in range(N_BLOCKS)]
            left = {s: N_BLOCKS for s in range(N_GROUPS)}
            for s, b in units:
                base, w, q = GROUP_BASES[s], GROUP_WIDTHS[s], GROUP_QUARTERS[s]
                if s not in ref_tiles:
                    ref_tiles[s] = ref_pool.tile(
                        [128, KSL, w], dt_mm, tag="ref", name=f"ref_sb{s}")
                    nc.gpsimd.memset(ref_tiles[s][:, KCH, :], 0)
                    for k in range(KCH):
                        nc.sync.dma_start(
                            ref_tiles[s][:, k, 0:w],
                            ref_d[k, :, base:base + w])
                ref_sb = ref_tiles[s]
                pt = psum_pool.tile([128, w], mybir.dt.float32,
                                    tag="pt", name=f"pt_{s}_{b}")
                off = 0
                for nj in GROUP_STRIPS[s]:
                    for p in range(3):
                        nc.tensor.matmul(
                            pt[:, off:off + nj],
                            inp_sb[:, 2 * p:2 * p + 2,
                                   b * 128:(b + 1) * 128],
                            ref_sb[:, 2 * p:2 * p + 2, off:off + nj],
                            start=(p == 0), stop=(p == 2),
                            perf_mode=DR)
                    off += nj
                pm = pm_pool.tile([128, GROUP_QUARTERS[0]], mybir.dt.float32)
                # DVE may read at most one PSUM operand per instruction, so
                # seed pm with a scalar-engine copy of quarter 0
                nc.scalar.copy(pm[:, :q], pt[:, 0:q])
                nc.vector.tensor_max(pm[:, :q], pm[:, :q], pt[:, q:2 * q])
                nc.vector.tensor_max(pm[:, :q], pm[:, :q], pt[:, 2 * q:3 * q])
                nc.vector.tensor_max(pm[:, :q], pm[:, :q], pt[:, 3 * q:w])
                slot = (s * N_BLOCKS + b) * 8
                nc.vector.max(acc_val[:, slot:slot + 8], pm[:, :q])
                nc.vector.max_index(acc_idx[:, slot:slot + 8],
                                    acc_val[:, slot:slot + 8],
                                    pm[:, :q])
                left[s] -= 1
                if left[s] == 0:
                    # stream this group's results out as soon as it is done
                    lo, hi = s * N_BLOCKS * 8, (s + 1) * N_BLOCKS * 8
                    nc.sync.dma_start(val_d[:, lo:hi], acc_val[:, lo:hi])
                    nc.sync.dma_start(idx_d[:, lo:hi], acc_idx[:, lo:hi])

    nc.compile()
    return nc


def _get_nc():
    if "nc" not in _COMPILED:
        _COMPILED["nc"] = _build_module()
    return _COMPILED["nc"]


def _unit_channels(f):
    # f: (N, C, H, W) float32; unit L2 norm over channels per pixel
    n = np.sqrt(np.sum(f * f, axis=1, keepdims=True, dtype=np.float32))
    return (f / np.maximum(n, EPS_NORMALIZE)).astype(np.float32)


def _patches(f):
    # f: (C, H, W) -> (K, NPATCH), row index = c*9 + dy*3 + dx
    out = np.empty((C, PS * PS, HP, WP), np.float32)
    for dy in range(PS):
        for dx in range(PS):
            out[:, dy * PS + dx] = f[:, dy:dy + HP, dx:dx + WP]
    return out.reshape(K, NPATCH)


def _prep_inputs(dense_features1, dense_features2):
    fi = _unit_channels(np.ascontiguousarray(dense_features1, np.float32))
    fr = _unit_channels(np.ascontiguousarray(dense_features2, np.float32))
    in_maps = []
    mats = []
    for n in range(N_ITEMS):
        inp_full = _patches(fi[n])                       # (576, 8836)
        ref_full = _patches(fr[n])                       # (576, 8836)
        mats.append((inp_full, ref_full))
        ref_pad = np.zeros((KPAD, NPATCH), np.float32)
        ref_pad[:K] = ref_full
        ref_pad = np.ascontiguousarray(
            ref_pad.reshape(KCH, 128, NPATCH)).astype(
                ml_dtypes.float8_e4m3fn)
        for j in range(CHUNKS_PER_ITEM):
            inp_pad = np.zeros((KPAD, CHUNK_PAD), np.float32)
            inp_pad[:K, :CHUNK] = inp_full[:, j * CHUNK:(j + 1) * CHUNK]
            inp_pad = np.ascontiguousarray(
                inp_pad.reshape(KCH, 128, CHUNK_PAD)).astype(
                    ml_dtypes.float8_e4m3fn)
            in_maps.append({"inp": inp_pad, "ref": ref_pad})
    return in_maps, mats


def _combine_core(val, idx):
    # val/idx: (128, N_GROUPS*N_BLOCKS*8), slot = (s*N_BLOCKS + b)*8
    # idx[..., 0] is the winning column's offset within its group quarter.
    # -> (CHUNK, 4) candidate global ref columns, (CHUNK,) cross-fold margin
    v8 = val.reshape(128, N_GROUPS, N_BLOCKS, 8)
    v8 = v8.transpose(2, 0, 1, 3).reshape(CHUNK_PAD, N_GROUPS * 8)[:CHUNK]
    v = v8[:, 0::8]                               # per-group top-1
    ix = idx.reshape(128, N_GROUPS, N_BLOCKS, 8)[..., 0].astype(np.int64)
    ix = ix.transpose(2, 0, 1).reshape(CHUNK_PAD, N_GROUPS)[:CHUNK]
    sel = np.argmax(v, axis=1)            # first occurrence == earliest group
    rows = np.arange(CHUNK)
    jA = ix[rows, sel]
    qs = np.asarray(GROUP_QUARTERS, dtype=np.int64)[sel]
    bases = np.asarray(GROUP_BASES, dtype=np.int64)[sel]
    cands = bases[:, None] + jA[:, None] + \
        qs[:, None] * np.arange(4, dtype=np.int64)[None, :]   # (CHUNK, 4)
    top2 = np.partition(v8, N_GROUPS * 8 - 2, axis=1)[:, -2:]
    margin = top2[:, 1] - top2[:, 0]
    return cands, margin


def _flow_output(max_idx):
    # max_idx: (NPATCH,) int -> (18, H, W) float32, mirroring the reference
    mi = max_idx.reshape(HP, WP)
    fw = (mi % WP).astype(np.float32) - np.arange(WP, dtype=np.float32)[None, :]
    fh = (mi // WP).astype(np.float32) - np.arange(HP, dtype=np.float32)[:, None]
    flow = np.stack([fw, fh], axis=-1)                     # (94, 94, 2)
    flow = np.pad(flow, ((0, PS - 1), (0, PS - 1), (0, 0)))  # (96, 96, 2)
    shifted = np.stack([np.pad(flow, ((i, 0), (j, 0), (0, 0)))[:H, :W]
                        for i in range(PS) for j in range(PS)], axis=0)
    out = np.stack([shifted[..., 1], shifted[..., 0]], axis=1)  # (9, 2, H, W)
    return out.reshape(2 * PS * PS, H, W).astype(np.float32)


def kernel(dense_features1, dense_features2):
    from concourse import bass_utils

    nc = _get_nc()
    in_maps, mats = _prep_inputs(dense_features1, dense_features2)
    res = bass_utils.run_bass_kernel_spmd(
        nc, in_maps, core_ids=list(range(N_CORES)))
    out = np.empty((N_ITEMS, 2 * PS * PS, H, W), np.float32)
    for n in range(N_ITEMS):
        parts = [
            _combine_core(res.results[n * CHUNKS_PER_ITEM + j]["val"],
                          res.results[n * CHUNKS_PER_ITEM + j]["idx"])
            for j in range(CHUNKS_PER_ITEM)
        ]
        cands = np.concatenate([p[0] for p in parts])      # (NPATCH, 4)
        margin = np.concatenate([p[1] for p in parts])
        inp_full, ref_full = mats[n]
        # resolve the window candidates with exact fp32 dot products
        g = ref_full[:, cands]                             # (576, NPATCH, 4)
        dots = np.einsum('kr,krq->rq', inp_full, g, optimize=True)
        max_idx = cands[np.arange(NPATCH), np.argmax(dots, axis=1)]
        flagged = np.flatnonzero(margin < MARGIN_THRESH)
        if flagged.size:
            # exact rescore of near-tie rows: fp32 sgemm first, fp64 only for
            # rows still ambiguous at fp32 rounding scale
            corr = inp_full[:, flagged].T @ ref_full
            max_idx[flagged] = np.argmax(corr, axis=1)
            top2 = np.partition(corr, corr.shape[1] - 2, axis=1)[:, -2:]
            risky = np.flatnonzero(top2[:, 1] - top2[:, 0] < 1e-3)
            if risky.size:
                corr64 = inp_full[:, flagged[risky]].T.astype(np.float64) @ \
                    ref_full.astype(np.float64)
                max_idx[flagged[risky]] = np.argmax(corr64, axis=1)
        out[n] = _flow_output(max_idx)
    return out



# revision 2
# speedup vs baseline: 1.2496x; 1.2496x over previous
"""CorrespondenceGeneration kernel for 8 TRN2 NeuronCores.

Reference computation (per item): unit-normalize features over channels,
build 3x3 patch matrices, corr = inp_patches^T @ ref_patches, argmax over
ref patches (first occurrence on ties), convert argmax index to flow,
9 tensor-shifts, channel reorder.

Sharding: core c -> (item = c//4, n_in chunk = c%4 of 2209 rows). Each core
computes its corr rows against ALL ref patches, streamed in 5 column groups
(widths 2048,2048,2048,2048,644 -- exactly 8836 real columns).

Engine split per (block, group) unit:
  - Tensor: K=576 split as 2 fp8-e4m3 DoubleRow matmuls (256 K-rows each)
    + 1 plain fp8 matmul for the 64-row tail (no DR penalty there).
  - Scalar: one ACTIVATE copies the left half of the PSUM tile to SBUF
    (cast to bf16).
  - Vector: ONE tensor_max folds the right half into it (fold-by-2).
  - DMA: the folded bf16 half-tile streams straight to HBM.
No on-device argmax at all: the host scans the folded values (4418 per
input row), picks the winning fold pair, and resolves its 2 members with
exact fp32 dot products. Rows whose cross-pair device margin is below
MARGIN_THRESH (fp8 matmul + bf16 rounding error scale) get a full exact
rescore on the host.

Note: the reference's per-patch-column normalization of ref divides every
column by ||col||+eps with ||col|| == 3 exactly (9 unit-norm pixels), a
global positive scale that argmax is invariant to -- so it is skipped.
"""

import sys

if "/opt/trn_rl_repo" not in sys.path:
    sys.path.insert(0, "/opt/trn_rl_repo")

import numpy as np
import ml_dtypes

# ---- problem constants (hardcoded; kernel.py must be self-contained) ----
N_ITEMS = 2
C = 64
H = W = 96
PS = 3
HP = WP = H - PS + 1          # 94
NPATCH = HP * WP              # 8836
K = C * PS * PS               # 576
KPAD = 640                    # 5 x 128
KCH = 5                       # K chunks of 128 (chunk 4 = 64 real + 64 zero)
N_CORES = 8
CHUNKS_PER_ITEM = 4
CHUNK = NPATCH // CHUNKS_PER_ITEM      # 2209
CHUNK_PAD = 2304                       # 18 x 128
N_BLOCKS = CHUNK_PAD // 128            # 18
# ref column groups: exactly the 8836 real columns
GROUP_BASES = (0, 2048, 4096, 6144, 8192)
GROUP_WIDTHS = (2048, 2048, 2048, 2048, 644)
GROUP_HALVES = tuple(w // 2 for w in GROUP_WIDTHS)     # 1024,...,322
N_GROUPS = len(GROUP_BASES)
# matmul strip widths per group (PSUM bank = 512 fp32; a matmul output must
# not cross a bank boundary, so strips are 512-aligned)
GROUP_STRIPS = ((512, 512, 512, 512),) * 4 + ((512, 132),)
# group-major offsets of each group's folded output in the val tensor
GROUP_VAL_OFF = tuple(
    sum(N_BLOCKS * h for h in GROUP_HALVES[:s]) for s in range(N_GROUPS))
VAL_W = sum(N_BLOCKS * h for h in GROUP_HALVES)        # 79524
EPS_NORMALIZE = 1e-12

# fp8-e4m3 matmul error (sigma ~1.4e-2) + bf16 fold rounding (~4e-3).
# Rows whose device cross-pair top-2 margin is below this get an exact host
# rescore.
MARGIN_THRESH = 0.08

_COMPILED = {}


def _build_module():
    import concourse.bacc as bacc
    from concourse.tile import TileContext
    from concourse import mybir

    dt_mm = mybir.dt.float8e4
    nc = bacc.Bacc("TRN2", target_bir_lowering=False, debug=False,
                   num_devices=N_CORES)
    inp_d = nc.dram_tensor("inp", [KCH, 128, CHUNK_PAD], dt_mm,
                           kind="ExternalInput").ap()
    ref_d = nc.dram_tensor("ref", [KCH, 128, NPATCH], dt_mm,
                           kind="ExternalInput").ap()
    val_d = nc.dram_tensor("val", [128, VAL_W], mybir.dt.bfloat16,
                           kind="ExternalOutput").ap()

    DR = mybir.MatmulPerfMode.DoubleRow

    with TileContext(nc) as tc:
        with tc.tile_pool(name="inp", bufs=1) as inp_pool, \
             tc.tile_pool(name="ref", bufs=2) as ref_pool, \
             tc.tile_pool(name="pm", bufs=4) as pm_pool, \
             tc.tile_pool(name="psum", bufs=2, space="PSUM") as psum_pool:
            # startup DMA order mirrors first-group matmul order so the first
            # matmuls are gated on ~100KB of DMA: strip-major for group 0.
            ref_tiles = {}
            w0 = GROUP_WIDTHS[0]
            ref_tiles[0] = ref_pool.tile([128, KCH, w0], dt_mm,
                                         tag="ref", name="ref_sb0")
            inp_sb = inp_pool.tile([128, KCH, CHUNK_PAD], dt_mm)
            for k in range(KCH):
                nc.sync.dma_start(inp_sb[:, k, 0:128], inp_d[k, :, 0:128])
            for j in range(w0 // 512):
                for k in range(KCH):
                    nc.sync.dma_start(
                        ref_tiles[0][:, k, j * 512:(j + 1) * 512],
                        ref_d[k, :, j * 512:(j + 1) * 512])
            # rest of inp (group 0 iterates over all blocks, so all of inp is
            # needed early)
            for (lo, hi) in [(128, 384), (384, 896), (896, 1664),
                             (1664, CHUNK_PAD)]:
                for k in range(KCH):
                    nc.sync.dma_start(inp_sb[:, k, lo:hi], inp_d[k, :, lo:hi])

            units = [(s, b) for s in range(N_GROUPS)
                     for b in range(N_BLOCKS)]
            for s, b in units:
                base, w, h = GROUP_BASES[s], GROUP_WIDTHS[s], GROUP_HALVES[s]
                if s not in ref_tiles:
                    ref_tiles[s] = ref_pool.tile(
                        [128, KCH, w], dt_mm, tag="ref", name=f"ref_sb{s}")
                    for k in range(KCH):
                        nc.sync.dma_start(
                            ref_tiles[s][:, k, 0:w],
                            ref_d[k, :, base:base + w])
                ref_sb = ref_tiles[s]
                pt = psum_pool.tile([128, w], mybir.dt.float32,
                                    tag="pt", name=f"pt_{s}_{b}")
                bcol = slice(b * 128, (b + 1) * 128)
                off = 0
                for nj in GROUP_STRIPS[s]:
                    st = slice(off, off + nj)
                    nc.tensor.matmul(
                        pt[:, st], inp_sb[:, 0:2, bcol], ref_sb[:, 0:2, st],
                        start=True, stop=False, perf_mode=DR)
                    nc.tensor.matmul(
                        pt[:, st], inp_sb[:, 2:4, bcol], ref_sb[:, 2:4, st],
                        start=False, stop=False, perf_mode=DR)
                    nc.tensor.matmul(
                        pt[:, st], inp_sb[:, 4, bcol], ref_sb[:, 4, st],
                        start=False, stop=True)
                    off += nj
                # fold-by-2 straight out of PSUM: scalar seeds the left half
                # (DVE may read at most one PSUM operand per instruction),
                # DVE maxes in the right half; both cast to bf16.
                pm = pm_pool.tile([128, GROUP_HALVES[0]], mybir.dt.bfloat16)
                nc.scalar.copy(pm[:, :h], pt[:, 0:h])
                nc.vector.tensor_max(pm[:, :h], pm[:, :h], pt[:, h:w])
                lo = GROUP_VAL_OFF[s] + b * h
                nc.sync.dma_start(val_d[:, lo:lo + h], pm[:, :h])

    nc.compile()
    return nc


def _get_nc():
    if "nc" not in _COMPILED:
        _COMPILED["nc"] = _build_module()
    return _COMPILED["nc"]


def _unit_channels(f):
    # f: (N, C, H, W) float32; unit L2 norm over channels per pixel
    n = np.sqrt(np.sum(f * f, axis=1, keepdims=True, dtype=np.float32))
    return (f / np.maximum(n, EPS_NORMALIZE)).astype(np.float32)


def _patches(f):
    # f: (C, H, W) -> (K, NPATCH), row index = c*9 + dy*3 + dx
    out = np.empty((C, PS * PS, HP, WP), np.float32)
    for dy in range(PS):
        for dx in range(PS):
            out[:, dy * PS + dx] = f[:, dy:dy + HP, dx:dx + WP]
    return out.reshape(K, NPATCH)


def _prep_inputs(dense_features1, dense_features2):
    fi = _unit_channels(np.ascontiguousarray(dense_features1, np.float32))
    fr = _unit_channels(np.ascontiguousarray(dense_features2, np.float32))
    in_maps = []
    mats = []
    for n in range(N_ITEMS):
        inp_full = _patches(fi[n])                       # (576, 8836)
        ref_full = _patches(fr[n])                       # (576, 8836)
        mats.append((inp_full, ref_full))
        ref_pad = np.zeros((KPAD, NPATCH), np.float32)
        ref_pad[:K] = ref_full
        ref_pad = np.ascontiguousarray(
            ref_pad.reshape(KCH, 128, NPATCH)).astype(
                ml_dtypes.float8_e4m3fn)
        for j in range(CHUNKS_PER_ITEM):
            inp_pad = np.zeros((KPAD, CHUNK_PAD), np.float32)
            inp_pad[:K, :CHUNK] = inp_full[:, j * CHUNK:(j + 1) * CHUNK]
            inp_pad = np.ascontiguousarray(
                inp_pad.reshape(KCH, 128, CHUNK_PAD)).astype(
                    ml_dtypes.float8_e4m3fn)
            in_maps.append({"inp": inp_pad, "ref": ref_pad})
    return in_maps, mats


# pm column offsets of each group within a row's concatenated fold stream
_PM_STARTS = np.cumsum((0,) + GROUP_HALVES[:-1]).astype(np.int64)
_PM_TOTAL = int(sum(GROUP_HALVES))                      # 4418


def _combine_core(val):
    # val: (128, VAL_W) bf16, group-major slots of folded maxima.
    # -> (CHUNK, 2) candidate global ref columns, (CHUNK,) cross-pair margin
    v = np.asarray(val).astype(np.float32)
    segs = []
    for s in range(N_GROUPS):
        h = GROUP_HALVES[s]
        g = v[:, GROUP_VAL_OFF[s]:GROUP_VAL_OFF[s] + N_BLOCKS * h]
        g = g.reshape(128, N_BLOCKS, h).transpose(1, 0, 2).reshape(
            CHUNK_PAD, h)
        segs.append(g[:CHUNK])
    pm = np.concatenate(segs, axis=1)                    # (CHUNK, 4418)
    j = np.argmax(pm, axis=1)
    rows = np.arange(CHUNK)
    top2 = np.partition(pm, _PM_TOTAL - 2, axis=1)[:, -2:]
    margin = top2[:, 1] - top2[:, 0]
    sel = np.searchsorted(_PM_STARTS, j, side="right") - 1
    jloc = j - _PM_STARTS[sel]
    bases = np.asarray(GROUP_BASES, dtype=np.int64)[sel]
    halves = np.asarray(GROUP_HALVES, dtype=np.int64)[sel]
    cands = np.stack([bases + jloc, bases + jloc + halves], axis=1)
    return cands, margin


def _flow_output(max_idx):
    # max_idx: (NPATCH,) int -> (18, H, W) float32, mirroring the reference
    mi = max_idx.reshape(HP, WP)
    fw = (mi % WP).astype(np.float32) - np.arange(WP, dtype=np.float32)[None, :]
    fh = (mi // WP).astype(np.float32) - np.arange(HP, dtype=np.float32)[:, None]
    flow = np.stack([fw, fh], axis=-1)                     # (94, 94, 2)
    flow = np.pad(flow, ((0, PS - 1), (0, PS - 1), (0, 0)))  # (96, 96, 2)
    shifted = np.stack([np.pad(flow, ((i, 0), (j, 0), (0, 0)))[:H, :W]
                        for i in range(PS) for j in range(PS)], axis=0)
    out = np.stack([shifted[..., 1], shifted[..., 0]], axis=1)  # (9, 2, H, W)
    return out.reshape(2 * PS * PS, H, W).astype(np.float32)


def kernel(dense_features1, dense_features2):
    from concourse import bass_utils

    nc = _get_nc()
    in_maps, mats = _prep_inputs(dense_features1, dense_features2)
    res = bass_utils.run_bass_kernel_spmd(
        nc, in_maps, core_ids=list(range(N_CORES)))
    out = np.empty((N_ITEMS, 2 * PS * PS, H, W), np.float32)
    for n in range(N_ITEMS):
        parts = [
            _combine_core(res.results[n * CHUNKS_PER_ITEM + j]["val"])
            for j in range(CHUNKS_PER_ITEM)
        ]
        cands = np.concatenate([p[0] for p in parts])      # (NPATCH, 2)
        margin = np.concatenate([p[1] for p in parts])
        inp_full, ref_full = mats[n]
        # resolve the pair candidates with exact fp32 dot products
        g = ref_full[:, cands]                             # (576, NPATCH, 2)
        dots = np.einsum('kr,krq->rq', inp_full, g, optimize=True)
        max_idx = cands[np.arange(NPATCH), np.argmax(dots, axis=1)]
        flagged = np.flatnonzero(margin < MARGIN_THRESH)
        if flagged.size:
            # exact rescore of near-tie rows: fp32 sgemm first, fp64 only for
            # rows still ambiguous at fp32 rounding scale
            corr = inp_full[:, flagged].T @ ref_full
            max_idx[flagged] = np.argmax(corr, axis=1)
            top2 = np.partition(corr, corr.shape[1] - 2, axis=1)[:, -2:]
            risky = np.flatnonzero(top2[:, 1] - top2[:, 0] < 1e-3)
            if risky.size:
                corr64 = inp_full[:, flagged[risky]].T.astype(np.float64) @ \
                    ref_full.astype(np.float64)
                max_idx[flagged[risky]] = np.argmax(corr64, axis=1)
        out[n] = _flow_output(max_idx)
    return out


# revision 6
# speedup vs baseline: 1.2534x; 1.0030x over previous
"""CorrespondenceGeneration kernel for 8 TRN2 NeuronCores.

Reference computation (per item): unit-normalize features over channels,
build 3x3 patch matrices, corr = inp_patches^T @ ref_patches, argmax over
ref patches (first occurrence on ties), convert argmax index to flow,
9 tensor-shifts, channel reorder.

Sharding: core c -> (item = c//4, n_in chunk = c%4 of 2209 rows). Each core
computes its corr rows against ALL ref patches, streamed in 5 column groups
(widths 2048,2048,2048,2048,644 -- exactly 8836 real columns).

Engine split per (block, group) unit:
  - Tensor: K=576 split as 2 fp8-e4m3 DoubleRow matmuls (256 K-rows each)
    + 1 plain fp8 matmul for the 64-row tail. Tail matmuls of adjacent
    strips are packed into disjoint PE row-groups (partitions 0-63 vs
    64-127, tail rows host-duplicated in chunk 4) so the two K=64 matmuls
    run concurrently in the array.
  - Scalar: one ACTIVATE copies the left half of the PSUM tile to SBUF
    (cast to bf16).
  - Vector: ONE tensor_max folds the right half into it (fold-by-2).
  - DMA: the folded bf16 half-tile streams straight to HBM via the
    gpsimd (SWDGE) queue, keeping the sync HWDGE ring free for the
    batched input loads (one DMA per k-chunk per ref group).
No on-device argmax at all: the host scans the folded values (4418 per
input row), picks the winning fold pair, and resolves its 2 members with
exact fp32 dot products. Rows whose cross-pair device margin is below
MARGIN_THRESH (fp8 matmul + bf16 rounding error scale) get a full exact
rescore on the host.

Note: the reference's per-patch-column normalization of ref divides every
column by ||col||+eps with ||col|| == 3 exactly (9 unit-norm pixels), a
global positive scale that argmax is invariant to -- so it is skipped.
"""

import sys

if "/opt/trn_rl_repo" not in sys.path:
    sys.path.insert(0, "/opt/trn_rl_repo")

import numpy as np
import ml_dtypes

# ---- problem constants (hardcoded; kernel.py must be self-contained) ----
N_ITEMS = 2
C = 64
H = W = 96
PS = 3
HP = WP = H - PS + 1          # 94
NPATCH = HP * WP              # 8836
K = C * PS * PS               # 576
KPAD = 640                    # 5 x 128
KCH = 5                       # K chunks of 128 (chunk 4 = 64 real + 64 zero)
N_CORES = 8
CHUNKS_PER_ITEM = 4
CHUNK = NPATCH // CHUNKS_PER_ITEM      # 2209
CHUNK_PAD = 2304                       # 18 x 128
N_BLOCKS = CHUNK_PAD // 128            # 18
# ref column groups: exactly the 8836 real columns
GROUP_BASES = (0, 2048, 4096, 6144, 8192)
GROUP_WIDTHS = (2048, 2048, 2048, 2048, 644)
GROUP_HALVES = tuple(w // 2 for w in GROUP_WIDTHS)     # 1024,...,322
N_GROUPS = len(GROUP_BASES)
# matmul strip widths per group (PSUM bank = 512 fp32; a matmul output must
# not cross a bank boundary, so strips are 512-aligned)
GROUP_STRIPS = ((512, 512, 512, 512),) * 4 + ((512, 132),)
# group-major offsets of each group's folded output in the val tensor
GROUP_VAL_OFF = tuple(
    sum(N_BLOCKS * h for h in GROUP_HALVES[:s]) for s in range(N_GROUPS))
VAL_W = sum(N_BLOCKS * h for h in GROUP_HALVES)        # 79524
EPS_NORMALIZE = 1e-12

# fp8-e4m3 matmul error (sigma ~1.4e-2) + bf16 fold rounding (~4e-3).
# Rows whose device cross-pair top-2 margin is below this get an exact host
# rescore.
MARGIN_THRESH = 0.08

_COMPILED = {}


def _build_module():
    import concourse.bacc as bacc
    from concourse.tile import TileContext
    from concourse import mybir

    dt_mm = mybir.dt.float8e4
    nc = bacc.Bacc("TRN2", target_bir_lowering=False, debug=False,
                   num_devices=N_CORES)
    inp_d = nc.dram_tensor("inp", [KCH, 128, CHUNK_PAD], dt_mm,
                           kind="ExternalInput").ap()
    ref_d = nc.dram_tensor("ref", [KCH, 128, NPATCH], dt_mm,
                           kind="ExternalInput").ap()
    val_d = nc.dram_tensor("val", [128, VAL_W], mybir.dt.bfloat16,
                           kind="ExternalOutput").ap()

    DR = mybir.MatmulPerfMode.DoubleRow

    with TileContext(nc) as tc:
        with tc.tile_pool(name="inp", bufs=1) as inp_pool, \
             tc.tile_pool(name="ref", bufs=3) as ref_pool, \
             tc.tile_pool(name="pm", bufs=4) as pm_pool, \
             tc.tile_pool(name="psum", bufs=2, space="PSUM") as psum_pool:
            # Batched input DMA: one transfer per k-chunk (HWDGE issue is
            # serialized per engine at ~0.6us each, so few+large wins).
            # inp goes on the scalar queue, ref on sync, so the two HWDGE
            # rings fill in parallel; k0/k1 first since the first DoubleRow
            # matmul only needs those.
            ref_tiles = {}
            w0 = GROUP_WIDTHS[0]
            ref_tiles[0] = ref_pool.tile([128, KCH, w0], dt_mm,
                                         tag="ref", name="ref_sb0")
            inp_sb = inp_pool.tile([128, KCH, CHUNK_PAD], dt_mm)
            for k in range(KCH):
                nc.scalar.dma_start(inp_sb[:, k, :], inp_d[k, :, :])
                nc.sync.dma_start(ref_tiles[0][:, k, 0:w0],
                                  ref_d[k, :, 0:w0])

            units = [(s, b) for s in range(N_GROUPS)
                     for b in range(N_BLOCKS)]
            for s, b in units:
                base, w, h = GROUP_BASES[s], GROUP_WIDTHS[s], GROUP_HALVES[s]
                if s not in ref_tiles:
                    ref_tiles[s] = ref_pool.tile(
                        [128, KCH, w], dt_mm, tag="ref", name=f"ref_sb{s}")
                    for k in range(KCH):
                        nc.sync.dma_start(
                            ref_tiles[s][:, k, 0:w],
                            ref_d[k, :, base:base + w])
                ref_sb = ref_tiles[s]
                pt = psum_pool.tile([128, w], mybir.dt.float32,
                                    tag="pt", name=f"pt_{s}_{b}")
                bcol = slice(b * 128, (b + 1) * 128)
                strips = GROUP_STRIPS[s]
                offs = [sum(strips[:i]) for i in range(len(strips))]
                for p0 in range(0, len(strips), 2):
                    sl = [slice(offs[p0 + i], offs[p0 + i] + strips[p0 + i])
                          for i in range(2)]
                    for st in sl:
                        nc.tensor.matmul(
                            pt[:, st], inp_sb[:, 0:2, bcol],
                            ref_sb[:, 0:2, st],
                            start=True, stop=False, perf_mode=DR)
                        nc.tensor.matmul(
                            pt[:, st], inp_sb[:, 2:4, bcol],
                            ref_sb[:, 2:4, st],
                            start=False, stop=False, perf_mode=DR)
                    # K=64 tails of the two strips, packed into disjoint PE
                    # row groups (rows 0-63 / 64-127) -> run concurrently
                    nc.tensor.matmul(
                        pt[:, sl[0]], inp_sb[0:64, 4, bcol],
                        ref_sb[0:64, 4, sl[0]],
                        start=False, stop=True)
                    nc.tensor.matmul(
                        pt[:, sl[1]], inp_sb[64:128, 4, bcol],
                        ref_sb[64:128, 4, sl[1]],
                        start=False, stop=True)
                # fold-by-2 straight out of PSUM: scalar seeds the left half
                # (DVE may read at most one PSUM operand per instruction),
                # DVE maxes in the right half; both cast to bf16.
                pm = pm_pool.tile([128, GROUP_HALVES[0]], mybir.dt.bfloat16)
                nc.scalar.copy(pm[:, :h], pt[:, 0:h])
                nc.vector.tensor_max(pm[:, :h], pm[:, :h], pt[:, h:w])
                lo = GROUP_VAL_OFF[s] + b * h
                nc.gpsimd.dma_start(val_d[:, lo:lo + h], pm[:, :h])

    nc.compile()
    return nc


def _get_nc():
    if "nc" not in _COMPILED:
        _COMPILED["nc"] = _build_module()
    return _COMPILED["nc"]


def _unit_channels(f):
    # f: (N, C, H, W) float32; unit L2 norm over channels per pixel
    n = np.sqrt(np.sum(f * f, axis=1, keepdims=True, dtype=np.float32))
    return (f / np.maximum(n, EPS_NORMALIZE)).astype(np.float32)


def _patches(f):
    # f: (C, H, W) -> (K, NPATCH), row index = c*9 + dy*3 + dx
    out = np.empty((C, PS * PS, HP, WP), np.float32)
    for dy in range(PS):
        for dx in range(PS):
            out[:, dy * PS + dx] = f[:, dy:dy + HP, dx:dx + WP]
    return out.reshape(K, NPATCH)


def _prep_inputs(dense_features1, dense_features2):
    fi = _unit_channels(np.ascontiguousarray(dense_features1, np.float32))
    fr = _unit_channels(np.ascontiguousarray(dense_features2, np.float32))
    in_maps = []
    mats = []
    for n in range(N_ITEMS):
        inp_full = _patches(fi[n])                       # (576, 8836)
        ref_full = _patches(fr[n])                       # (576, 8836)
        mats.append((inp_full, ref_full))
        ref_pad = np.zeros((KPAD, NPATCH), np.float32)
        ref_pad[:K] = ref_full
        ref_pad[K:KPAD] = ref_full[K - (KPAD - K):]   # duplicate K=64 tail
        ref_pad = np.ascontiguousarray(
            ref_pad.reshape(KCH, 128, NPATCH)).astype(
                ml_dtypes.float8_e4m3fn)
        for j in range(CHUNKS_PER_ITEM):
            inp_pad = np.zeros((KPAD, CHUNK_PAD), np.float32)
            inp_pad[:K, :CHUNK] = inp_full[:, j * CHUNK:(j + 1) * CHUNK]
            inp_pad[K:KPAD] = inp_pad[2 * K - KPAD:K]  # duplicate K=64 tail
            inp_pad = np.ascontiguousarray(
                inp_pad.reshape(KCH, 128, CHUNK_PAD)).astype(
                    ml_dtypes.float8_e4m3fn)
            in_maps.append({"inp": inp_pad, "ref": ref_pad})
    return in_maps, mats


# pm column offsets of each group within a row's concatenated fold stream
_PM_STARTS = np.cumsum((0,) + GROUP_HALVES[:-1]).astype(np.int64)
_PM_TOTAL = int(sum(GROUP_HALVES))                      # 4418


def _combine_core(val):
    # val: (128, VAL_W) bf16, group-major slots of folded maxima.
    # -> (CHUNK, 2) candidate global ref columns, (CHUNK,) cross-pair margin
    v = np.asarray(val).astype(np.float32)
    segs = []
    for s in range(N_GROUPS):
        h = GROUP_HALVES[s]
        g = v[:, GROUP_VAL_OFF[s]:GROUP_VAL_OFF[s] + N_BLOCKS * h]
        g = g.reshape(128, N_BLOCKS, h).transpose(1, 0, 2).reshape(
            CHUNK_PAD, h)
        segs.append(g[:CHUNK])
    pm = np.concatenate(segs, axis=1)                    # (CHUNK, 4418)
    j = np.argmax(pm, axis=1)
    rows = np.arange(CHUNK)
    top2 = np.partition(pm, _PM_TOTAL - 2, axis=1)[:, -2:]
    margin = top2[:, 1] - top2[:, 0]
    sel = np.searchsorted(_PM_STARTS, j, side="right") - 1
    jloc = j - _PM_STARTS[sel]
    bases = np.asarray(GROUP_BASES, dtype=np.int64)[sel]
    halves = np.asarray(GROUP_HALVES, dtype=np.int64)[sel]
    cands = np.stack([bases + jloc, bases + jloc + halves], axis=1)
    return cands, margin


def _flow_output(max_idx):
    # max_idx: (NPATCH,) int -> (18, H, W) float32, mirroring the reference
    mi = max_idx.reshape(HP, WP)
    fw = (mi % WP).astype(np.float32) - np.arange(WP, dtype=np.float32)[None, :]
    fh = (mi // WP).astype(np.float32) - np.arange(HP, dtype=np.float32)[:, None]
    flow = np.stack([fw, fh], axis=-1)                     # (94, 94, 2)
    flow = np.pad(flow, ((0, PS - 1), (0, PS - 1), (0, 0)))  # (96, 96, 2)
    shifted = np.stack([np.pad(flow, ((i, 0), (j, 0), (0, 0)))[:H, :W]
                        for i in range(PS) for j in range(PS)], axis=0)
    out = np.stack([shifted[..., 1], shifted[..., 0]], axis=1)  # (9, 2, H, W)
    return out.reshape(2 * PS * PS, H, W).astype(np.float32)


def kernel(dense_features1, dense_features2):
    from concourse import bass_utils

    nc = _get_nc()
    in_maps, mats = _prep_inputs(dense_features1, dense_features2)
    res = bass_utils.run_bass_kernel_spmd(
        nc, in_maps, core_ids=list(range(N_CORES)))
    out = np.empty((N_ITEMS, 2 * PS * PS, H, W), np.float32)
    for n in range(N_ITEMS):
        parts = [
            _combine_core(res.results[n * CHUNKS_PER_ITEM + j]["val"])
            for j in range(CHUNKS_PER_ITEM)
        ]
        cands = np.concatenate([p[0] for p in parts])      # (NPATCH, 2)
        margin = np.concatenate([p[1] for p in parts])
        inp_full, ref_full = mats[n]
        # resolve the pair candidates with exact fp32 dot products
        g = ref_full[:, cands]                             # (576, NPATCH, 2)
        dots = np.einsum('kr,krq->rq', inp_full, g, optimize=True)
        max_idx = cands[np.arange(NPATCH), np.argmax(dots, axis=1)]
        flagged = np.flatnonzero(margin < MARGIN_THRESH)
        if flagged.size:
            # exact rescore of near-tie rows: fp32 sgemm first, fp64 only for
            # rows still ambiguous at fp32 rounding scale
            corr = inp_full[:, flagged].T @ ref_full
            max_idx[flagged] = np.argmax(corr, axis=1)
            top2 = np.partition(corr, corr.shape[1] - 2, axis=1)[:, -2:]
            risky = np.flatnonzero(top2[:, 1] - top2[:, 0] < 1e-3)
            if risky.size:
                corr64 = inp_full[:, flagged[risky]].T.astype(np.float64) @ \
                    ref_full.astype(np.float64)
                max_idx[flagged[risky]] = np.argmax(corr64, axis=1)
        out[n] = _flow_output(max_idx)
    return out
